# revision 42
# baseline (speedup 1.0000x reference)
"""AttentionBlock (GroupNorm -> 1x1 qkv -> full self-attention -> out-proj -> residual)
on Trainium2, data-parallel over batch across 8 NeuronCores.

Full input shapes (hardcoded):
  x        (32, 256, 32, 32) fp32
  gn_weight(256,) gn_bias (256,)
  w_qkv    (768, 256)  b_qkv (768,)
  w_out    (256, 256)  b_out (256,)

Per-core work: 4 batch elements. All large matmuls run in fp8e4 with
perf_mode=DoubleRow (contraction 256 in one pass at 0.5 cycles/row).

Key structural folds (host-side):
  * M = 16*Wk^T Wq replaces q,k: S^T = xn^T M xn. The per-t bias
    16*(Wk^T bq).xn folds into the exp bias; per-s terms cancel in softmax.
  * Wout folds into V: attention averages positions, Wout mixes channels,
    so Wout(attn.V) = attn.(Wout Wv xn) =: attn.v2 -- the out-projection
    matmul AND its PSUM evacuation disappear. The softmax division
    po[o,s]/po2[s] then directly produces the final (pre-residual) output.
  * bo = b_out + Wout b_v rides the residual: host DMAs xr = x + bo (bf16).
    Row-wise GN variance is invariant to the per-channel shift; row means
    are corrected by -bo before group aggregation, and the xn-apply bias
    gets the (mean_g + bo) version. y = po/po2 + xr is then exact.
  * ones weights = 16.0 so po2 = 16*D matches po = P*(16*v2): y = po/po2.

I/O is bf16 (host casts): halves DMA traffic; out is cast back to fp32.

GroupNorm rsqrt = exp(-0.5*ln(var+eps)); the activation-table map is patched
at build time so Exp and Ln both resolve to natural_log_exp_and_others and
ACT never reloads its function table.

The schedule is engine-balanced and software-pipelined at batch scope:
  ACT : 16 exp evacs, GN rsqrt, z-evac m=0 (3 of 4 batches)
  DVE : GN stats, z/v evacs, beta, softmax reciprocal + divide
  Pool: GN scalar chain + xn apply (next batch), residual adds
  PE  : everything matmul at fp8 DoubleRow rates
"""

import functools
import numpy as np

NCORES = 8
B, C, H, W = 32, 256, 32, 32
HW = H * W
BPC = B // NCORES        # batches per core
G = 8                    # groups
GSZ = C // G             # 32 channels / group
EPS = 1e-5
CT = C // 128            # channel tiles = 2
TT = HW // 128           # position tiles = 8
NT = HW // 512           # free-dim (512) tiles = 2

SC = 16.0                # host weight scale
EXP_SCALE = 1.0 / (SC * 16.0)        # 1/256: undo the 16 on M, apply 1/sqrt(c)
EXP_BIAS = -2.0
ONES_VAL = 16.0          # denominator scale: po2 = 16*D matches P*(16*v2)

# packed fp8 weight tensor layout (elements per partition)
_W8_M = 0            # [2, 256]  (M^T packed)
_W8_WV2 = 512        # [2, 256]  (16*(Wout Wv)^T)
_W8_ONES = 1024      # [2, 128] all ONES_VAL (DoubleRow denominator)
_W8_WVEC = 1280      # [2, 1]   16*Wk^T bq
_W8_TOT = 1282
# packed fp32 scalar tensor (GN indicators + biases), single DMA
_S_IND1 = 0          # 8
_S_IND2 = 8          # rows 0-3: 128
_S_BO = 136          # 2  (b_out + w_out@b_v per kt; GN mean corrections)
_S_GNWB = 138        # [2, 2]
_S_MBO = 142         # 2  group-mean of bo, per channel
_S_SBO2 = 144        # 2  group-mean of bo^2, per channel
_S_TOT = 146

_LOOP_N = 1


def _patch_act_tables():
    """Restrict Exp/Ln to the one table that holds both, so the act-table
    insertion pass cannot thrash between exp_and_others and natural_log."""
    import concourse.hw_specs as hs
    import concourse.bacc as bacc_mod
    from concourse import mybir

    if getattr(hs, "_attn_fp8_tbl_patch", False):
        return
    orig = hs.get_activation_tables

    @functools.cache
    def patched(arch):
        AF = mybir.ActivationFunctionType
        out = {}
        for name, fns in orig(arch).items():
            fns = set(fns)
            if name != "natural_log_exp_and_others":
                fns.discard(AF.Exp)
                fns.discard(AF.Ln)
            out[name] = fns
        return out

    hs.get_activation_tables = patched
    for mod in (bacc_mod,):
        if getattr(mod, "get_activation_tables", None) is orig:
            mod.get_activation_tables = patched
    hs._attn_fp8_tbl_patch = True


@functools.lru_cache(maxsize=None)
def _build(loop_n: int):
    import concourse.bacc as bacc
    import concourse.tile as tile
    from concourse import bass, mybir

    _patch_act_tables()

    f32 = mybir.dt.float32
    bf16 = mybir.dt.bfloat16
    f8 = mybir.dt.float8e4
    AF = mybir.ActivationFunctionType
    OP = mybir.AluOpType
    DR = mybir.MatmulPerfMode.DoubleRow

    nc = bacc.Bacc("TRN2", target_bir_lowering=False, debug=False)

    x_d = nc.declare_dram_parameter("x", [BPC, 128, CT * HW], bf16, isOutput=False)
    parw8_d = nc.declare_dram_parameter("parw8", [128, _W8_TOT], f8, isOutput=False)
    pars_d = nc.declare_dram_parameter("pars", [128, _S_TOT], f32, isOutput=False)
    out_d = nc.declare_dram_parameter("out", [BPC, 128, CT * HW], bf16, isOutput=True)

    with tile.TileContext(nc) as tc:
        with (
            nc.allow_low_precision(reason="fp8 DoubleRow matmul pipeline by design"),
            tc.tile_pool(name="const", bufs=1) as const,
            tc.tile_pool(name="xnp", bufs=2) as xnp,
            tc.tile_pool(name="qkp", bufs=2) as qkp,
            tc.tile_pool(name="vp", bufs=2) as vp,
            tc.tile_pool(name="ptp", bufs=2) as ptp,
            tc.tile_pool(name="outp", bufs=3) as outp,
            tc.tile_pool(name="statp", bufs=2) as statp,
            tc.tile_pool(name="rbp", bufs=2) as rbp,
            tc.tile_pool(name="pS", bufs=2, space="PSUM") as pS,     # [128,512]
            tc.tile_pool(name="pF", bufs=1, space="PSUM") as pF,     # [128,2,512]
            tc.tile_pool(name="pob", bufs=1, space="PSUM") as pob,   # o2+od tags
            tc.tile_pool(name="pgn", bufs=1, space="PSUM") as pgn,
        ):
            # ---- packed constants ----
            parw8_sb = const.tile([128, _W8_TOT], f8, name="parw8_sb")
            pars_sb = const.tile([128, _S_TOT], f32, name="pars_sb")
            m8_sb = parw8_sb[:, _W8_M : _W8_M + 512].rearrange("p (k f) -> p k f", f=256)
            wv2_sb = parw8_sb[:, _W8_WV2 : _W8_WV2 + 512].rearrange("p (k f) -> p k f", f=256)
            ones_sb = parw8_sb[:, _W8_ONES : _W8_ONES + 256].rearrange("p (k f) -> p k f", f=128)
            wvec_sb = parw8_sb[:, _W8_WVEC : _W8_WVEC + 2].rearrange("p (k f) -> p k f", f=1)
            ind1_sb = pars_sb[:, _S_IND1 : _S_IND1 + 8]
            ind2_sb = pars_sb[0:4, _S_IND2 : _S_IND2 + 128]
            bo_sb = pars_sb[:, _S_BO : _S_BO + 2].rearrange("p (k j) -> p k j", j=1)
            gnwb_sb = pars_sb[:, _S_GNWB : _S_GNWB + 4].rearrange("p (k j) -> p k j", j=2)
            mbo_sb = pars_sb[:, _S_MBO : _S_MBO + 2].rearrange("p (k j) -> p k j", j=1)
            sbo2_sb = pars_sb[:, _S_SBO2 : _S_SBO2 + 2].rearrange("p (k j) -> p k j", j=1)
            eps_sb = const.tile([128, 1], f32, name="eps_sb")
            nc.vector.memset(eps_sb, EPS)

            # loop_n <= 8: python-unrolled; loop_n > 8: hardware For_i loop
            unroll, hw_loop = (loop_n, 1) if loop_n <= 8 else (1, loop_n)

            # Fixed-address state for the ROTATED batch 0: the body's bottom
            # (during batch 3's attention) prepares next iteration's batch-0
            # GN/front into these, so a For_i iteration never stalls on the
            # serial GroupNorm->z chain at its head. All x tiles are fixed
            # and refilled (same bytes) each iteration so the two-ahead
            # bn_stats prefetch can read across the For_i edge.
            xts = [
                const.tile([128, CT * HW], bf16, name=f"x{b}_sb")
                for b in range(BPC)
            ]
            xn0_sb = const.tile([128, CT, HW], f8, name="xn0_sb")
            z0_sb = const.tile([128, CT, HW], f8, name="z0_sb")
            v0_sb = const.tile([128, TT, 256], f8, name="v0_sb")
            beta0_sb = const.tile([128, TT], f32, name="beta0_sb")

            gn = {}   # per-batch GN state
            qks, vs = {}, {}
            seq = [0]

            def _u():
                seq[0] += 1
                return str(seq[0])

            if True:
                def emit_gn_stats(b):
                    """bn_stats/bn_aggr on DVE (first, ungated work)."""
                    u = f"{b}_{_u()}"
                    xv = xts[b].rearrange("p (k f) -> p k f", f=HW)
                    mv = statp.tile([128, CT, 2], f32, name=f"mv_{u}", tag="mv")
                    for kt in range(CT):
                        bnst = statp.tile([128, 2, 6], f32, name=f"bn_{u}_{kt}", tag="bnst")
                        xq = xv[:, kt, :].rearrange("p (a c) -> p a c", c=512)
                        for sg in range(2):
                            nc.vector.bn_stats(out=bnst[:, sg, :], in_=xq[:, sg, :])
                        nc.vector.bn_aggr(out=mv[:, kt, :], in_=bnst)
                    gn[b] = {"mv": mv}

                def emit_gn_mid(b):
                    """Row-mean bo-correction (Pool), indicator matmuls +
                    group aggregation (PE), scalar chain (Pool/DVE)."""
                    u = f"{b}_{_u()}"
                    mv = gn[b]["mv"]
                    s12 = statp.tile([128, CT, 4], f32, name=f"s12_{u}", tag="s12")
                    pg = pgn.tile([4, 2 * 4], f32, name=f"pg_{u}", tag="gn")
                    # raw rows are xr = x + bo: row var is bo-invariant, row
                    # mean needs -bo before group aggregation
                    nc.gpsimd.tensor_sub(s12[:, :, 0:1], mv[:, :, 0:1], bo_sb)
                    nc.gpsimd.tensor_copy(out=s12[:, :, 1:2], in_=mv[:, :, 1:2])
                    nc.gpsimd.tensor_copy(out=s12[:, :, 3:4], in_=s12[:, :, 0:1])
                    nc.gpsimd.tensor_mul(s12[:, :, 2:3], s12[:, :, 0:1], s12[:, :, 0:1])
                    for kt in range(CT):
                        nc.tensor.matmul(
                            pg[:, 4 * kt : 4 * kt + 4],
                            ind1_sb[:, 4 * kt : 4 * kt + 4],
                            s12[:, kt, :],
                        )
                    gsum = statp.tile([4, 8], f32, name=f"gs_{u}", tag="gs")
                    nc.vector.tensor_copy(out=gsum, in_=pg)
                    ps2 = pgn.tile([128, CT, 4], f32, name=f"ps2_{u}", tag="gn")
                    for kt in range(CT):
                        nc.tensor.matmul(
                            ps2[:, kt, :], ind2_sb, gsum[:, 4 * kt : 4 * kt + 4]
                        )
                    # ms = [mean_g, E[var], E[mean^2], pad]; var = ms1+ms2-ms0^2
                    ms = statp.tile([128, CT, 4], f32, name=f"ms_{u}", tag="ms")
                    nc.vector.tensor_scalar_mul(out=ms, in0=ps2, scalar1=1.0 / GSZ)
                    mx = statp.tile([128, CT, 1], f32, name=f"mx_{u}", tag="mx")
                    va = statp.tile([128, CT, 1], f32, name=f"va_{u}", tag="va")
                    tmp = statp.tile([128, CT, 1], f32, name=f"tmp_{u}", tag="tmp")
                    nc.gpsimd.tensor_copy(out=mx, in_=ms[:, :, 0:1])
                    nc.gpsimd.tensor_add(va, ms[:, :, 1:2], ms[:, :, 2:3])
                    nc.gpsimd.tensor_mul(tmp, mx, mx)
                    nc.gpsimd.tensor_sub(va, va, tmp)
                    gn[b].update(mx=mx, va=va, tmp=tmp)

                def emit_gn_act(b):
                    """rsqrt via exp(-0.5*ln(var+eps)) on ACT, then the
                    per-channel scale/bias and the xn apply (Pool). The
                    xn-apply runs on xr, so its bias uses mean_g + bo."""
                    u = f"{b}_{_u()}"
                    d = gn[b]
                    mx, va, tmp = d["mx"], d["va"], d["tmp"]
                    xv = xts[b].rearrange("p (k f) -> p k f", f=HW)
                    rs = statp.tile([128, CT, 1], f32, name=f"rs_{u}", tag="rs")
                    nc.scalar.activation(out=va, in_=va, func=AF.Ln, bias=eps_sb)
                    nc.scalar.activation(out=rs, in_=va, func=AF.Exp, scale=-0.5)
                    ab = statp.tile([128, CT, 2], f32, name=f"ab_{u}", tag="ab")
                    mb = statp.tile([128, CT, 1], f32, name=f"mb_{u}", tag="mb")
                    nc.gpsimd.tensor_mul(ab[:, :, 0:1], gnwb_sb[:, :, 0:1], rs)
                    nc.gpsimd.tensor_add(mb, mx, bo_sb)
                    nc.gpsimd.tensor_mul(tmp, mb, ab[:, :, 0:1])
                    nc.gpsimd.tensor_sub(ab[:, :, 1:2], gnwb_sb[:, :, 1:2], tmp)
                    xn_sb = xn0_sb if b == 0 else xnp.tile(
                        [128, CT, HW], f8, name=f"xn_{u}", tag="xn"
                    )
                    for kt in range(CT):
                        nc.gpsimd.tensor_scalar(
                            out=xn_sb[:, kt, :],
                            in0=xv[:, kt, :],
                            scalar1=ab[:, kt, 0:1],
                            scalar2=ab[:, kt, 1:2],
                            op0=OP.mult,
                            op1=OP.add,
                        )
                    gn[b]["xn"] = xn_sb

                def emit_z_m(b, m, zact=False, new=False):
                    """One m-half of z = M xn (2 matmuls + one evac)."""
                    u = f"{b}_{_u()}"
                    xn_sb = gn[b]["xn"]
                    if new:
                        qks[b] = z0_sb if b == 0 else qkp.tile(
                            [128, CT, HW], f8, name=f"z_{u}", tag="z"
                        )
                    z_sb = qks[b]
                    pz = pF.tile([128, 2, 512], f32, name=f"pz_{u}_{m}", tag="f")
                    for n in range(NT):
                        nc.tensor.matmul(
                            pz[:, n, :],
                            m8_sb[:, :, 128 * m : 128 * m + 128],
                            xn_sb[:, :, 512 * n : 512 * n + 512],
                            start=True, stop=True, perf_mode=DR,
                        )
                    if zact:
                        nc.scalar.activation(
                            out=z_sb[:, m, :],
                            in_=pz.rearrange("p k f -> p (k f)"),
                            func=AF.Copy,
                        )
                    else:
                        nc.vector.tensor_copy(
                            out=z_sb[:, m, :],
                            in_=pz.rearrange("p k f -> p (k f)"),
                        )

                def emit_beta(b):
                    """beta[t] = (16 Wk^T bq).xn_t * EXP_SCALE + EXP_BIAS (DVE)."""
                    u = f"{b}_{_u()}"
                    xn_sb = gn[b]["xn"]
                    pb = pgn.tile([128, TT], f32, name=f"pb_{u}", tag="gn")
                    for t in range(TT):
                        nc.tensor.matmul(
                            pb[:, t : t + 1],
                            xn_sb[:, :, 128 * t : 128 * t + 128],
                            wvec_sb,
                            start=True, stop=True, perf_mode=DR,
                        )
                    beta = beta0_sb if b == 0 else statp.tile(
                        [128, TT], f32, name=f"beta_{u}", tag="beta"
                    )
                    nc.vector.tensor_scalar(
                        out=beta, in0=pb, scalar1=EXP_SCALE, scalar2=EXP_BIAS,
                        op0=OP.mult, op1=OP.add,
                    )
                    gn[b]["beta"] = beta

                def emit_v_g2(b, g2):
                    """One 4-t-tile half of v2T = xn^T (Wout Wv)T."""
                    u = f"{b}_{_u()}"
                    xn_sb = gn[b]["xn"]
                    if g2 == 0:
                        vs[b] = v0_sb if b == 0 else vp.tile(
                            [128, TT, 256], f8, name=f"v_{u}", tag="v"
                        )
                    v_sb = vs[b]
                    pv = pF.tile([128, 2, 512], f32, name=f"pv_{u}_{g2}", tag="f")
                    for gq in range(2):
                        for tq in range(2):
                            t = 4 * g2 + 2 * gq + tq
                            nc.tensor.matmul(
                                pv[:, gq, 256 * tq : 256 * tq + 256],
                                xn_sb[:, :, 128 * t : 128 * t + 128],
                                wv2_sb,
                                start=True, stop=True, perf_mode=DR,
                            )
                    nc.vector.tensor_copy(
                        out=v_sb[:, 4 * g2 : 4 * g2 + 4, :].rearrange(
                            "p k f -> p (k f)"
                        ),
                        in_=pv.rearrange("p k f -> p (k f)"),
                    )

                def emit_prologue():
                    for b in range(BPC):
                        nc.sync.dma_start(out=xts[b], in_=x_d[b])
                        if b == 0:
                            nc.sync.dma_start(out=pars_sb, in_=pars_d[:, :])
                        if b == 1:
                            nc.sync.dma_start(out=parw8_sb, in_=parw8_d[:, :])
                    emit_gn_stats(0)
                    emit_gn_mid(0)
                    emit_gn_act(0)
                    emit_z_m(0, 0, new=True)
                    emit_z_m(0, 1)
                    emit_beta(0)
                    emit_v_g2(0, 0)
                    emit_v_g2(0, 1)

                def emit_body():
                  for b in range(1, BPC):
                    nc.sync.dma_start(out=xts[b], in_=x_d[b])
                  for b in range(BPC):
                    u = f"{b}_{_u()}"
                    z_sb = qks[b]
                    xn_b = gn[b]["xn"]
                    beta_b = gn[b]["beta"]
                    v_sb = vs[b]
                    xv_b = xts[b].rearrange("p (k f) -> p k f", f=HW)
                    nxt = (b + 1) % BPC
                    if b + 1 == BPC:
                        # refill x0 (same bytes) and rotate next iteration's
                        # batch-0 prep under this batch's attention
                        nc.sync.dma_start(out=xts[0], in_=x_d[0])
                    emit_gn_stats(nxt)
                    y_sb = outp.tile([128, CT, HW], bf16, name=f"y_{u}", tag="y")
                    pts = ptp.tile([128, TT, HW], f8, name=f"pT_{u}", tag="pT")
                    po = {}

                    def alloc_o(n):
                        po[0, n] = pob.tile(
                            [128, 2, 512], f32, name=f"po_{u}_{n}", tag="o2"
                        )
                        po[2, n] = pob.tile([128, 512], f32, name=f"pd_{u}_{n}", tag="od")

                    def emit_o(gp, n):
                        st, sp = (gp == 0), (gp == TT // 2 - 1)
                        rhs = pts[:, 2 * gp : 2 * gp + 2, 512 * n : 512 * n + 512]
                        nc.tensor.matmul(po[0, n][:, 0, :], v_sb[:, 2 * gp : 2 * gp + 2, 0:128],
                                         rhs, start=st, stop=sp, perf_mode=DR)
                        nc.tensor.matmul(po[0, n][:, 1, :], v_sb[:, 2 * gp : 2 * gp + 2, 128:256],
                                         rhs, start=st, stop=sp, perf_mode=DR)
                        nc.tensor.matmul(po[2, n], ones_sb,
                                         rhs, start=st, stop=sp, perf_mode=DR)

                    def s_exp(t, n):
                        psT = pS.tile([128, 512], f32, name=f"pS_{u}_{t}_{n}", tag="s")
                        nc.tensor.matmul(
                            psT,
                            xn_b[:, :, 128 * t : 128 * t + 128],
                            z_sb[:, :, 512 * n : 512 * n + 512],
                            start=True, stop=True, perf_mode=DR,
                        )
                        nc.scalar.activation(
                            out=pts[:, t, 512 * n : 512 * n + 512], in_=psT,
                            func=AF.Exp, scale=EXP_SCALE, bias=beta_b[:, t : t + 1],
                        )

                    def normalize(n):
                        """y[:, :, n-half] = po[:,n] / po2[n] (final output
                        sans residual), one mul with rb broadcast over kt."""
                        rb = rbp.tile([128, 512], f32, name=f"rb_{u}_{n}", tag="rb")
                        nc.vector.reciprocal_approx_fast(out=rb, in_=po[2, n])
                        pb01, rbb = bass.broadcast_tensor_aps(
                            po[0, n], rb.rearrange("p (k f) -> p k f", k=1)
                        )
                        nc.vector.tensor_mul(
                            y_sb[:, :, 512 * n : 512 * n + 512], pb01, rbb
                        )

                    # All of batch nxt's prep interleaves into this batch's
                    # exp stream, each producer landing a half-batch before
                    # its consumer: GN chain at t1/t2 (stats prefetched two
                    # batches ahead), z halves at t4/t6 (evacs ahead of
                    # normalize in the DVE queue), beta/v in the n=1 loop.
                    alloc_o(0)
                    for t in range(TT):
                        s_exp(t, 0)
                        if t == 2:
                            emit_gn_mid(nxt)
                        if t == 4:
                            emit_gn_act(nxt)
                        if t >= 3 and t % 2 == 1:
                            emit_o((t - 3) // 2, 0)
                    emit_o(TT // 2 - 1, 0)
                    # z(nxt) evacs go ahead of normalize in the DVE queue so
                    # they run while this batch's exps stream on ACT
                    emit_z_m(nxt, 0, zact=True, new=True)
                    emit_z_m(nxt, 1)
                    emit_beta(nxt)
                    normalize(0)
                    alloc_o(1)
                    for t in range(TT):
                        s_exp(t, 1)
                        if t == 2:
                            emit_v_g2(nxt, 0)
                        if t == 4:
                            emit_v_g2(nxt, 1)
                        if t >= 3 and t % 2 == 1:
                            emit_o((t - 3) // 2, 1)
                    emit_o(TT // 2 - 1, 1)
                    normalize(1)
                    # residual (+bo, folded into xr on host) and store
                    for kt in range(CT):
                        nc.gpsimd.tensor_add(
                            y_sb[:, kt, :], y_sb[:, kt, :], xv_b[:, kt, :]
                        )
                    nc.sync.dma_start(
                        out=out_d[b][:, :],
                        in_=y_sb.rearrange("p k f -> p (k f)"),
                    )

                emit_prologue()
                if hw_loop == 1:
                    for _ in range(unroll):
                        emit_body()
                else:
                    with tc.For_i(0, hw_loop, 1):
                        emit_body()
    nc.compile()
    return nc


def _host_inputs(x, gn_weight, gn_bias, w_qkv, b_qkv, w_out, b_out):
    """Fold/reshape parameters into the packed layout; shard x."""
    import ml_dtypes

    f = np.float32
    f8 = ml_dtypes.float8_e4m3fn
    bf = ml_dtypes.bfloat16
    x = np.ascontiguousarray(x, dtype=f).reshape(B, C, HW)
    wq = w_qkv[0:256].astype(f)
    wk = w_qkv[256:512].astype(f)
    wv = w_qkv[512:768].astype(f)
    wo = w_out.astype(f)
    m8T = (f(SC) * (wk.T @ wq)).T                           # (256, 256): z = M xn
    wv2T = (wo @ wv).T * f(SC)                              # (256, 256)
    bq = b_qkv[0:256].astype(f)
    bv = b_qkv[512:768].astype(f)
    wvec = f(SC) * (wk.T @ bq)                              # (256,)
    bo = b_out.astype(f) + wo @ bv                          # (256,)

    parw8 = np.zeros((128, _W8_TOT), dtype=f8)
    pars = np.zeros((128, _S_TOT), dtype=f)
    for kt in range(CT):
        sl = slice(128 * kt, 128 * kt + 128)
        parw8[:, _W8_M + 256 * kt : _W8_M + 256 * kt + 256] = m8T[sl].astype(f8)
        parw8[:, _W8_WV2 + 256 * kt : _W8_WV2 + 256 * kt + 256] = wv2T[sl].astype(f8)
        parw8[:, _W8_ONES + 128 * kt : _W8_ONES + 128 * kt + 128] = f8(ONES_VAL)
        parw8[:, _W8_WVEC + kt] = wvec[sl].astype(f8)
        pars[:, _S_BO + kt] = bo[sl]
        pars[:, _S_GNWB + 2 * kt] = gn_weight.astype(f)[sl]
        pars[:, _S_GNWB + 2 * kt + 1] = gn_bias.astype(f)[sl]
    grp = bo.reshape(G, GSZ)
    mbo = np.repeat(grp.mean(axis=1), GSZ)                  # (256,)
    sbo2 = np.repeat((grp * grp).mean(axis=1), GSZ)         # (256,)
    for kt in range(CT):
        sl = slice(128 * kt, 128 * kt + 128)
        pars[:, _S_MBO + kt] = mbo[sl]
        pars[:, _S_SBO2 + kt] = sbo2[sl]
    for gl in range(4):
        pars[32 * gl : 32 * gl + 32, _S_IND1 + gl] = 1.0
        pars[32 * gl : 32 * gl + 32, _S_IND1 + 4 + gl] = 1.0
    for cc in range(128):
        pars[cc // 32, _S_IND2 + cc] = 1.0

    xr = x + bo[None, :, None]                              # residual carries bo
    in_maps = []
    for i in range(NCORES):
        xs = xr[BPC * i : BPC * (i + 1)].reshape(BPC, CT, 128, HW)
        xs = np.ascontiguousarray(
            xs.transpose(0, 2, 1, 3).reshape(BPC, 128, CT * HW)
        ).astype(bf)
        in_maps.append({"x": xs, "parw8": parw8, "pars": pars})
    return in_maps


def kernel(x, gn_weight, gn_bias, w_qkv, b_qkv, w_out, b_out):
    from concourse.bass_utils import run_bass_kernel_spmd

    in_maps = _host_inputs(x, gn_weight, gn_bias, w_qkv, b_qkv, w_out, b_out)
    nc = _build(_LOOP_N)
    res = run_bass_kernel_spmd(nc, in_maps, list(range(NCORES)))
    outs = []
    for i in range(NCORES):
        o = res.results[i]["out"].astype(np.float32).reshape(BPC, 128, CT, HW)
        outs.append(o.transpose(0, 2, 1, 3).reshape(BPC, C, HW))
    return np.concatenate(outs).reshape(B, C, H, W).astype(np.float32)
